# revision 82
# baseline (speedup 1.0000x reference)
"""Trainium2 Bass kernel for the masked-MSE actor-critic criterion.

Problem: inputs sample_seq/sample_value/sample_reward, all [65536, 256].
  mask[i, j] = 1 iff no zero appears in sample_seq[i, :j]  (prefix property)
  loss       = sum((reward-value)^2 * mask) / sum(mask)
  returns (loss, mean(reward-value), mean(reward))

Strategy (pure data-parallel over 8 NeuronCores):
  - Host shards the batch dim 8 ways and TRANSPOSES each shard to [S=256, 8192]
    so the sequence dim lies along SBUF partitions (2 blocks of 128).
  - seq ships as uint8 (values 0..19, lossless), reward/value as bf16
    (perturbs the loss by ~3e-5 relative; tolerance is 2e-2).
  - Per DMA tile of RD=2048 batch columns x 2 seq blocks:
      g  = (seq == 0)              VectorE tensor_scalar (2x_2p mode)
      C0 = Tri^T @ g0              TensorE per 512-col PSUM chunk
      C1 = Tri^T @ g1 + Ones128^T @ g0   (all-ones lhsT broadcasts block0's
                                   zero-count to every row -- no copies)
      mask = relu(1 - C)           ScalarE, PSUM -> SBUF bf16,
                                   accum_out -> per-chunk sum(mask)
      d  = r - v; dk = d * mask    VectorE tensor_tensor (2x_1p mode)
      dm = dk * dk                 (= d^2 * mask since mask is 0/1)
      sum(r), sum(d), sum(dm)      TensorE ones-matmuls, PSUM-accumulated
                                   across the whole kernel
  - Outputs per core: "sums" [1, 3*512] raw per-column stat partials and
    "acc" [128, nmask] mask sums. Host sums the 8 cores' partials in f64
    and forms the 3 outputs.
"""

import numpy as np

B, S = 65536, 256
N_CORES = 8
P = 128
COLS = B // N_CORES  # 8192 columns (batch rows) per core
RD = 2048            # columns per DMA tile
R = 512              # columns per PSUM chunk (one bank / one matmul)

_cache = {}


def build_nc(cols, rd=RD, r=R, seq_dtype="u8", taper=False, gsplit=False,
             front=True, dmsplit="mod3", cpair=False, pairio=False,
             pmode="stack", early_stats=True):
    from concourse import bacc, tile, mybir

    dt = mybir.dt
    assert cols % rd == 0 and rd % r == 0
    ndma = cols // rd
    nchunk = rd // r
    # per-chunk mask-sum columns (cpair merges both seq blocks per chunk)
    nmask = (cols // r) * (1 if cpair else 2)
    rc = 2048                    # dk/dm compute-chunk size

    if taper:
        # big tiles early (few DMA issues), small at the end so the
        # trailing compute after the last DMA is short
        widths = [rd] * (ndma - 1) + [rd // 2, rd // 4, rd // 4]
    elif front:
        # small tiles first so compute starts sooner
        widths = [rd // 4, rd // 4, rd // 2] + [rd] * (ndma - 1)
    else:
        widths = [rd] * ndma
    assert sum(widths) == cols
    # pre-enumerate dm instances (tile, block, chunk) and route each to
    # ScalarE (Square + accum col) or VectorE (dk*dk + PE sums)
    def dm_route(i, b):
        if not dmsplit:
            return False
        if dmsplit == "b1":
            return b == 1
        if dmsplit == "mod2":
            return i % 2 == 0
        if dmsplit == "mod3":
            return i % 3 == 2
        raise ValueError(dmsplit)

    dm_insts = []
    i = 0
    for w in widths:
        for b in range(2):
            for cc in range((w + rc - 1) // rc):
                rct = min(rc, w - cc * rc)
                dm_insts.append(dm_route(i, b))
                i += 1
    ndmcols = sum(dm_insts)
    nacc = nmask + ndmcols

    nc = bacc.Bacc("TRN2", target_bir_lowering=False, debug=False,
                   num_devices=N_CORES)

    seq_dt = dt.uint8 if seq_dtype == "u8" else dt.bfloat16
    seq_d = nc.declare_dram_parameter("seq", [S, cols], seq_dt, isOutput=False)
    rew_d = nc.declare_dram_parameter("rew", [S, cols], dt.bfloat16, isOutput=False)
    val_d = nc.declare_dram_parameter("val", [S, cols], dt.bfloat16, isOutput=False)
    tri_d = nc.declare_dram_parameter("tri", [P, P], dt.bfloat16, isOutput=False)
    onesm_d = nc.declare_dram_parameter("onesm", [P, P], dt.bfloat16, isOutput=False)
    ones_d = nc.declare_dram_parameter("ones", [P, 1], dt.bfloat16, isOutput=False)
    sums_d = nc.declare_dram_parameter("sums", [1, 3 * r], dt.float32,
                                       isOutput=True)
    acc_d = nc.declare_dram_parameter("acc", [P, nacc], dt.float32, isOutput=True)

    AT = mybir.ActivationFunctionType
    OP = mybir.AluOpType
    NSEG = 3  # psum stat segments: 0=r, 1=d, 2=dm
    nch_all = (cols // r) * 2
    seg2 = 0
    i = 0
    for w in widths:
        for b in range(2):
            for cc in range((w + rc - 1) // rc):
                rct = min(rc, w - cc * rc)
                if not dm_insts[i]:
                    seg2 += rct // r
                i += 1
    seg_totals = {0: nch_all, 1: nch_all, 2: seg2}

    with tile.TileContext(nc, pool_alloc_mode=pmode) as tc:
        with (
            tc.tile_pool(name="const", bufs=1) as constp,
            tc.tile_pool(name="io", bufs=2) as iop,
            tc.tile_pool(name="mid", bufs=2) as midp,
            tc.tile_pool(name="accp", bufs=1) as accp,
            tc.tile_pool(name="cpsum", bufs=2, space="PSUM") as cpsump,
            tc.tile_pool(name="spsum", bufs=1, space="PSUM") as spsump,
            tc.tile_pool(name="outp", bufs=1) as outp,
        ):
            tri_t = constp.tile([P, P], dt.bfloat16)
            nc.sync.dma_start(tri_t[:], tri_d[:])
            onesm_t = constp.tile([P, P], dt.bfloat16)
            nc.sync.dma_start(onesm_t[:], onesm_d[:])
            ones_t = constp.tile([P, 1], dt.bfloat16)
            nc.sync.dma_start(ones_t[:], ones_d[:])

            acc = accp.tile([P, nacc], dt.float32, name="acc")
            stats = spsump.tile([1, NSEG * r], dt.float32)
            counts = {}
            dmcol = [0]
            dminst = [0]

            def pe_sum(iseg, rhs_ap):
                k = counts.get(iseg, 0)
                counts[iseg] = k + 1
                nc.tensor.matmul(stats[0:1, iseg * r:(iseg + 1) * r], ones_t[:],
                                 rhs_ap, start=(k == 0),
                                 stop=(k == seg_totals[iseg] - 1),
                                 skip_group_check=True)

            tiles = []
            pos = 0
            for w in widths:
                tiles.append((pos, w))
                pos += w

            chbase = 0
            for c0, rdt in tiles:
                nchunk_t = rdt // r
                maskp = midp.tile([P, 2, rdt], dt.bfloat16, tag="maskp",
                                  name="maskp")
                mv = lambda b, sl: maskp[:, b, sl]
                if pairio:
                    sqp = iop.tile([P, 2, rdt], seq_dt, tag="seqp", name="sqp")
                    rp = iop.tile([P, 2, rdt], dt.bfloat16, tag="rp", name="rp")
                    vp = iop.tile([P, 2, rdt], dt.bfloat16, tag="vp", name="vp")
                    for b in range(2):
                        pl, ph = b * P, (b + 1) * P
                        nc.sync.dma_start(sqp[:, b, :], seq_d[pl:ph, c0:c0 + rdt])
                        nc.sync.dma_start(rp[:, b, :], rew_d[pl:ph, c0:c0 + rdt])
                        nc.sync.dma_start(vp[:, b, :], val_d[pl:ph, c0:c0 + rdt])
                    gp = midp.tile([P, 2, rdt], dt.bfloat16, tag="gp", name="gp")
                    nc.vector.tensor_scalar(gp[:], sqp[:], 0.0, None,
                                            OP.is_equal)
                    gv = lambda b, sl: gp[:, b, sl]
                    rv = lambda b, sl: rp[:, b, sl]
                    vvv = lambda b, sl: vp[:, b, sl]
                else:
                    rs, vs, gs = [], [], []
                    for b in range(2):
                        pl, ph = b * P, (b + 1) * P
                        sq = iop.tile([P, rdt], seq_dt, tag=f"seq{b}",
                                      name=f"seq{b}")
                        rr = iop.tile([P, rdt], dt.bfloat16, tag=f"r{b}",
                                      name=f"r{b}")
                        vv = iop.tile([P, rdt], dt.bfloat16, tag=f"v{b}",
                                      name=f"v{b}")
                        nc.sync.dma_start(sq[:], seq_d[pl:ph, c0:c0 + rdt])
                        nc.sync.dma_start(rr[:], rew_d[pl:ph, c0:c0 + rdt])
                        nc.sync.dma_start(vv[:], val_d[pl:ph, c0:c0 + rdt])
                        rs.append(rr); vs.append(vv)

                        g = midp.tile([P, rdt], dt.bfloat16, tag=f"g{b}",
                                      name=f"g{b}")
                        if gsplit and b == 1:
                            # relu(1 - seq) == (seq == 0) for integer seq >= 0
                            nc.scalar.activation(g[:], sq[:], AT.Relu,
                                                 bias=1.0, scale=-1.0)
                        else:
                            nc.vector.tensor_scalar(g[:], sq[:], 0.0, None,
                                                    OP.is_equal)
                        gs.append(g)
                    gv = lambda b, sl: gs[b][:, sl]
                    rv = lambda b, sl: rs[b][:, sl]
                    vvv = lambda b, sl: vs[b][:, sl]

                # prefix zero-counts + masks, per 512-col PSUM chunk
                for ch in range(nchunk_t):
                    sl = slice(ch * r, (ch + 1) * r)
                    if cpair:
                        mcol = chbase + ch
                        cp = cpsump.tile([P, 2 * r], dt.float32, tag="cp")
                        nc.tensor.matmul(cp[:, 0:r], tri_t[:], gv(0, sl))
                        nc.tensor.matmul(cp[:, r:2 * r], tri_t[:], gv(1, sl),
                                         start=True, stop=False)
                        nc.tensor.matmul(cp[:, r:2 * r], onesm_t[:], gv(0, sl),
                                         start=False, stop=True)
                        # one relu covers both blocks; accum = chunk mask sum
                        nc.scalar.activation(
                            maskp[:, :, sl],
                            cp[:].rearrange("p (b r) -> p b r", b=2),
                            AT.Relu, bias=1.0, scale=-1.0,
                            accum_out=acc[:, mcol:mcol + 1])
                    else:
                        mcol = (chbase + ch) * 2
                        c0p = cpsump.tile([P, r], dt.float32, tag="c0p")
                        nc.tensor.matmul(c0p[:], tri_t[:], gv(0, sl))
                        c1p = cpsump.tile([P, r], dt.float32, tag="c1p")
                        nc.tensor.matmul(c1p[:], tri_t[:], gv(1, sl),
                                         start=True, stop=False)
                        nc.tensor.matmul(c1p[:], onesm_t[:], gv(0, sl),
                                         start=False, stop=True)
                        nc.scalar.activation(mv(0, sl), c0p[:], AT.Relu,
                                             bias=1.0, scale=-1.0,
                                             accum_out=acc[:, mcol:mcol + 1])
                        nc.scalar.activation(mv(1, sl), c1p[:], AT.Relu,
                                             bias=1.0, scale=-1.0,
                                             accum_out=acc[:, mcol + 1:mcol + 2])
                chbase += nchunk_t

                if pairio:
                    dp = midp.tile([P, 2, rdt], dt.bfloat16, tag="dp", name="dp")
                    nc.vector.tensor_tensor(dp[:], rp[:], vp[:], OP.subtract)
                for b in range(2):
                    if pairio:
                        dv = lambda sl: dp[:, b, sl]
                    else:
                        d = midp.tile([P, rdt], dt.bfloat16, tag=f"d{b}",
                                      name=f"d{b}")
                        nc.vector.tensor_tensor(d[:], rs[b][:], vs[b][:],
                                                OP.subtract)
                        dv = lambda sl: d[:, sl]
                    if early_stats:
                        # r/d sums have no mask dependency; emit them ahead
                        # of the mask-gated dk/dm chain
                        for ch in range(nchunk_t):
                            sl = slice(ch * r, (ch + 1) * r)
                            pe_sum(0, rv(b, sl))
                            pe_sum(1, dv(sl))
                    for cc in range((rdt + rc - 1) // rc):
                        rct = min(rc, rdt - cc * rc)
                        cs = slice(cc * rc, cc * rc + rct)
                        dk = midp.tile([P, rct], dt.bfloat16, tag=f"dk{b}",
                                       name=f"dk{b}")
                        nc.vector.tensor_tensor(dk[:], dv(cs), mv(b, cs),
                                                OP.mult)
                        dms = midp.tile([P, rct], dt.bfloat16, tag=f"dms{b}",
                                        name=f"dms{b}")
                        on_act = dm_insts[dminst[0]]
                        dminst[0] += 1
                        if on_act:
                            dc = dmcol[0]
                            dmcol[0] += 1
                            nc.scalar.activation(
                                dms[:], dk[:], AT.Square,
                                accum_out=acc[:, nmask + dc:nmask + dc + 1])
                        else:
                            nc.vector.tensor_tensor(dms[:], dk[:], dk[:],
                                                    OP.mult)
                        for ch in range(rct // r):
                            sl = slice(cc * rc + ch * r, cc * rc + (ch + 1) * r)
                            sl2 = slice(ch * r, (ch + 1) * r)
                            if not early_stats:
                                pe_sum(0, rv(b, sl))
                                pe_sum(1, dv(sl))
                            if not on_act:
                                pe_sum(2, dms[:, sl2])

            # ship raw per-column stat partials; the host sums 1536 floats
            sums_s = outp.tile([1, NSEG * r], dt.float32)
            nc.scalar.copy(sums_s[:], stats[:])
            nc.sync.dma_start(sums_d[:], sums_s[:])
            nc.sync.dma_start(acc_d[:], acc[:])

    nc.compile()
    meta = {"nmask": nmask, "seq_dtype": seq_dtype, "r": r}
    return nc, meta


def make_consts():
    import ml_dtypes
    bf16 = ml_dtypes.bfloat16
    # tri[k, j] = 1 if k < j  (strictly-lower prefix: C[j] = # zeros before j)
    tri = np.triu(np.ones((P, P), dtype=np.float32), 1).astype(bf16)
    onesm = np.ones((P, P), dtype=bf16)
    ones = np.ones((P, 1), dtype=bf16)
    return tri, onesm, ones


def prep_shards(sample_seq, sample_value, sample_reward, seq_dtype="u8"):
    """Host-side shard prep: batch-shard 8 ways, transpose to [S, cols]."""
    import ml_dtypes
    bf16 = ml_dtypes.bfloat16
    seq_np = np.uint8 if seq_dtype == "u8" else bf16
    seq_bf = np.asarray(sample_seq).astype(seq_np)        # values in [0, 20)
    rew_bf = np.asarray(sample_reward).astype(bf16)
    val_bf = np.asarray(sample_value).astype(bf16)

    tri, onesm, ones = make_consts()
    in_maps = []
    for c in range(N_CORES):
        lo, hi = c * COLS, (c + 1) * COLS
        in_maps.append({
            "seq": np.ascontiguousarray(seq_bf[lo:hi].T),
            "rew": np.ascontiguousarray(rew_bf[lo:hi].T),
            "val": np.ascontiguousarray(val_bf[lo:hi].T),
            "tri": tri,
            "onesm": onesm,
            "ones": ones,
        })
    return in_maps


def combine(parts, meta):
    """parts: per-core dicts with 'sums' [1, 3*R] (r/d/dm per-column partials)
    and 'acc' [P, nmask] mask sums."""
    sum_r = sum_mask = sum_d = sum_dm = 0.0
    rr = meta["r"]
    for p in parts:
        sums = np.asarray(p["sums"], dtype=np.float64)[0]
        sum_r += sums[0:rr].sum()
        sum_d += sums[rr:2 * rr].sum()
        acc = np.asarray(p["acc"], dtype=np.float64)
        sum_dm += sums[2 * rr:].sum() + acc[:, meta["nmask"]:].sum()
        sum_mask += acc[:, :meta["nmask"]].sum()
    n = float(B) * float(S)
    return np.array([sum_dm / sum_mask, sum_d / n, sum_r / n], dtype=np.float32)


def run(sample_seq, sample_value, sample_reward, trace=False, build_kwargs=None,
        **kwargs):
    from concourse.bass_utils import run_bass_kernel_spmd

    key = tuple(sorted((build_kwargs or {}).items()))
    if key not in _cache:
        _cache[key] = build_nc(COLS, **(build_kwargs or {}))
    nc, meta = _cache[key]

    in_maps = prep_shards(sample_seq, sample_value, sample_reward,
                          seq_dtype=meta["seq_dtype"])
    res = run_bass_kernel_spmd(nc, in_maps, core_ids=list(range(N_CORES)),
                               trace=trace, **kwargs)
    return combine(res.results, meta), res


def kernel(sample_seq, sample_value, sample_reward):
    out, _ = run(sample_seq, sample_value, sample_reward)
    return out


# revision 83
# speedup vs baseline: 1.0219x; 1.0219x over previous
"""Trainium2 Bass kernel for the masked-MSE actor-critic criterion.

Problem: inputs sample_seq/sample_value/sample_reward, all [65536, 256].
  mask[i, j] = 1 iff no zero appears in sample_seq[i, :j]  (prefix property)
  loss       = sum((reward-value)^2 * mask) / sum(mask)
  returns (loss, mean(reward-value), mean(reward))

Strategy (pure data-parallel over 8 NeuronCores):
  - Host shards the batch dim 8 ways and TRANSPOSES each shard to [S=256, 8192]
    so the sequence dim lies along SBUF partitions (2 blocks of 128).
  - seq ships as uint8 (values 0..19, lossless), reward/value as bf16
    (perturbs the loss by ~3e-5 relative; tolerance is 2e-2).
  - Per DMA tile of RD=2048 batch columns x 2 seq blocks:
      g  = (seq == 0)              VectorE tensor_scalar (2x_2p mode)
      C0 = Tri^T @ g0              TensorE per 512-col PSUM chunk
      C1 = Tri^T @ g1 + Ones128^T @ g0   (all-ones lhsT broadcasts block0's
                                   zero-count to every row -- no copies)
      mask = relu(1 - C)           ScalarE, PSUM -> SBUF bf16,
                                   accum_out -> per-chunk sum(mask)
      d  = r - v; dk = d * mask    VectorE tensor_tensor (2x_1p mode)
      dm = dk * dk                 (= d^2 * mask since mask is 0/1)
      sum(r), sum(d), sum(dm)      TensorE ones-matmuls, PSUM-accumulated
                                   across the whole kernel
  - Outputs per core: "sums" [1, 3*512] raw per-column stat partials and
    "acc" [128, nmask] mask sums. Host sums the 8 cores' partials in f64
    and forms the 3 outputs.
"""

import numpy as np

B, S = 65536, 256
N_CORES = 8
P = 128
COLS = B // N_CORES  # 8192 columns (batch rows) per core
RD = 2048            # columns per DMA tile
R = 512              # columns per PSUM chunk (one bank / one matmul)

_cache = {}


def build_nc(cols, rd=RD, r=R, seq_dtype="u8", taper=False, gsplit=False,
             front=True, dmsplit="mod3", cpair=False, pairio=False,
             pmode="stack", early_stats=True):
    from concourse import bacc, tile, mybir

    dt = mybir.dt
    assert cols % rd == 0 and rd % r == 0
    ndma = cols // rd
    nchunk = rd // r
    # per-chunk mask-sum columns (cpair merges both seq blocks per chunk)
    nmask = (cols // r) * (1 if cpair else 2)
    rc = 1024                    # dk/dm compute-chunk size

    if taper:
        # big tiles early (few DMA issues), small at the end so the
        # trailing compute after the last DMA is short
        widths = [rd] * (ndma - 1) + [rd // 2, rd // 4, rd // 4]
    elif front:
        # small tiles first so compute starts sooner
        widths = [rd // 4, rd // 4, rd // 2] + [rd] * (ndma - 1)
    else:
        widths = [rd] * ndma
    assert sum(widths) == cols
    # pre-enumerate dm instances (tile, block, chunk) and route each to
    # ScalarE (Square + accum col) or VectorE (dk*dk + PE sums)
    def dm_route(i, b):
        if not dmsplit:
            return False
        if dmsplit == "b1":
            return b == 1
        if dmsplit == "mod2":
            return i % 2 == 0
        if dmsplit == "mod3":
            return i % 3 == 2
        raise ValueError(dmsplit)

    dm_insts = []
    i = 0
    for w in widths:
        for b in range(2):
            for cc in range((w + rc - 1) // rc):
                rct = min(rc, w - cc * rc)
                dm_insts.append(dm_route(i, b))
                i += 1
    ndmcols = sum(dm_insts)
    nacc = nmask + ndmcols

    nc = bacc.Bacc("TRN2", target_bir_lowering=False, debug=False,
                   num_devices=N_CORES)

    seq_dt = dt.uint8 if seq_dtype == "u8" else dt.bfloat16
    seq_d = nc.declare_dram_parameter("seq", [S, cols], seq_dt, isOutput=False)
    rew_d = nc.declare_dram_parameter("rew", [S, cols], dt.bfloat16, isOutput=False)
    val_d = nc.declare_dram_parameter("val", [S, cols], dt.bfloat16, isOutput=False)
    tri_d = nc.declare_dram_parameter("tri", [P, P], dt.bfloat16, isOutput=False)
    onesm_d = nc.declare_dram_parameter("onesm", [P, P], dt.bfloat16, isOutput=False)
    ones_d = nc.declare_dram_parameter("ones", [P, 1], dt.bfloat16, isOutput=False)
    sums_d = nc.declare_dram_parameter("sums", [1, 3 * r], dt.float32,
                                       isOutput=True)
    acc_d = nc.declare_dram_parameter("acc", [P, nacc], dt.float32, isOutput=True)

    AT = mybir.ActivationFunctionType
    OP = mybir.AluOpType
    NSEG = 3  # psum stat segments: 0=r, 1=d, 2=dm
    nch_all = (cols // r) * 2
    seg2 = 0
    i = 0
    for w in widths:
        for b in range(2):
            for cc in range((w + rc - 1) // rc):
                rct = min(rc, w - cc * rc)
                if not dm_insts[i]:
                    seg2 += rct // r
                i += 1
    seg_totals = {0: nch_all, 1: nch_all, 2: seg2}

    with tile.TileContext(nc, pool_alloc_mode=pmode) as tc:
        with (
            tc.tile_pool(name="const", bufs=1) as constp,
            tc.tile_pool(name="io", bufs=2) as iop,
            tc.tile_pool(name="mid", bufs=2) as midp,
            tc.tile_pool(name="accp", bufs=1) as accp,
            tc.tile_pool(name="cpsum", bufs=2, space="PSUM") as cpsump,
            tc.tile_pool(name="spsum", bufs=1, space="PSUM") as spsump,
            tc.tile_pool(name="outp", bufs=1) as outp,
        ):
            tri_t = constp.tile([P, P], dt.bfloat16)
            nc.sync.dma_start(tri_t[:], tri_d[:])
            onesm_t = constp.tile([P, P], dt.bfloat16)
            nc.sync.dma_start(onesm_t[:], onesm_d[:])
            ones_t = constp.tile([P, 1], dt.bfloat16)
            nc.sync.dma_start(ones_t[:], ones_d[:])

            acc = accp.tile([P, nacc], dt.float32, name="acc")
            stats = spsump.tile([1, NSEG * r], dt.float32)
            counts = {}
            dmcol = [0]
            dminst = [0]

            def pe_sum(iseg, rhs_ap):
                k = counts.get(iseg, 0)
                counts[iseg] = k + 1
                nc.tensor.matmul(stats[0:1, iseg * r:(iseg + 1) * r], ones_t[:],
                                 rhs_ap, start=(k == 0),
                                 stop=(k == seg_totals[iseg] - 1),
                                 skip_group_check=True)

            tiles = []
            pos = 0
            for w in widths:
                tiles.append((pos, w))
                pos += w

            chbase = 0
            for c0, rdt in tiles:
                nchunk_t = rdt // r
                maskp = midp.tile([P, 2, rdt], dt.bfloat16, tag="maskp",
                                  name="maskp")
                mv = lambda b, sl: maskp[:, b, sl]
                if pairio:
                    sqp = iop.tile([P, 2, rdt], seq_dt, tag="seqp", name="sqp")
                    rp = iop.tile([P, 2, rdt], dt.bfloat16, tag="rp", name="rp")
                    vp = iop.tile([P, 2, rdt], dt.bfloat16, tag="vp", name="vp")
                    for b in range(2):
                        pl, ph = b * P, (b + 1) * P
                        nc.sync.dma_start(sqp[:, b, :], seq_d[pl:ph, c0:c0 + rdt])
                        nc.sync.dma_start(rp[:, b, :], rew_d[pl:ph, c0:c0 + rdt])
                        nc.sync.dma_start(vp[:, b, :], val_d[pl:ph, c0:c0 + rdt])
                    gp = midp.tile([P, 2, rdt], dt.bfloat16, tag="gp", name="gp")
                    nc.vector.tensor_scalar(gp[:], sqp[:], 0.0, None,
                                            OP.is_equal)
                    gv = lambda b, sl: gp[:, b, sl]
                    rv = lambda b, sl: rp[:, b, sl]
                    vvv = lambda b, sl: vp[:, b, sl]
                else:
                    rs, vs, gs = [], [], []
                    for b in range(2):
                        pl, ph = b * P, (b + 1) * P
                        sq = iop.tile([P, rdt], seq_dt, tag=f"seq{b}",
                                      name=f"seq{b}")
                        rr = iop.tile([P, rdt], dt.bfloat16, tag=f"r{b}",
                                      name=f"r{b}")
                        vv = iop.tile([P, rdt], dt.bfloat16, tag=f"v{b}",
                                      name=f"v{b}")
                        nc.sync.dma_start(sq[:], seq_d[pl:ph, c0:c0 + rdt])
                        nc.sync.dma_start(rr[:], rew_d[pl:ph, c0:c0 + rdt])
                        nc.sync.dma_start(vv[:], val_d[pl:ph, c0:c0 + rdt])
                        rs.append(rr); vs.append(vv)

                        g = midp.tile([P, rdt], dt.bfloat16, tag=f"g{b}",
                                      name=f"g{b}")
                        if gsplit and b == 1:
                            # relu(1 - seq) == (seq == 0) for integer seq >= 0
                            nc.scalar.activation(g[:], sq[:], AT.Relu,
                                                 bias=1.0, scale=-1.0)
                        else:
                            nc.vector.tensor_scalar(g[:], sq[:], 0.0, None,
                                                    OP.is_equal)
                        gs.append(g)
                    gv = lambda b, sl: gs[b][:, sl]
                    rv = lambda b, sl: rs[b][:, sl]
                    vvv = lambda b, sl: vs[b][:, sl]

                # prefix zero-counts + masks, per 512-col PSUM chunk
                for ch in range(nchunk_t):
                    sl = slice(ch * r, (ch + 1) * r)
                    if cpair:
                        mcol = chbase + ch
                        cp = cpsump.tile([P, 2 * r], dt.float32, tag="cp")
                        nc.tensor.matmul(cp[:, 0:r], tri_t[:], gv(0, sl))
                        nc.tensor.matmul(cp[:, r:2 * r], tri_t[:], gv(1, sl),
                                         start=True, stop=False)
                        nc.tensor.matmul(cp[:, r:2 * r], onesm_t[:], gv(0, sl),
                                         start=False, stop=True)
                        # one relu covers both blocks; accum = chunk mask sum
                        nc.scalar.activation(
                            maskp[:, :, sl],
                            cp[:].rearrange("p (b r) -> p b r", b=2),
                            AT.Relu, bias=1.0, scale=-1.0,
                            accum_out=acc[:, mcol:mcol + 1])
                    else:
                        mcol = (chbase + ch) * 2
                        c0p = cpsump.tile([P, r], dt.float32, tag="c0p")
                        nc.tensor.matmul(c0p[:], tri_t[:], gv(0, sl))
                        c1p = cpsump.tile([P, r], dt.float32, tag="c1p")
                        nc.tensor.matmul(c1p[:], tri_t[:], gv(1, sl),
                                         start=True, stop=False)
                        nc.tensor.matmul(c1p[:], onesm_t[:], gv(0, sl),
                                         start=False, stop=True)
                        nc.scalar.activation(mv(0, sl), c0p[:], AT.Relu,
                                             bias=1.0, scale=-1.0,
                                             accum_out=acc[:, mcol:mcol + 1])
                        nc.scalar.activation(mv(1, sl), c1p[:], AT.Relu,
                                             bias=1.0, scale=-1.0,
                                             accum_out=acc[:, mcol + 1:mcol + 2])
                chbase += nchunk_t

                if pairio:
                    dp = midp.tile([P, 2, rdt], dt.bfloat16, tag="dp", name="dp")
                    nc.vector.tensor_tensor(dp[:], rp[:], vp[:], OP.subtract)
                for b in range(2):
                    if pairio:
                        dv = lambda sl: dp[:, b, sl]
                    else:
                        d = midp.tile([P, rdt], dt.bfloat16, tag=f"d{b}",
                                      name=f"d{b}")
                        nc.vector.tensor_tensor(d[:], rs[b][:], vs[b][:],
                                                OP.subtract)
                        dv = lambda sl: d[:, sl]
                    if early_stats:
                        # r/d sums have no mask dependency; emit them ahead
                        # of the mask-gated dk/dm chain
                        for ch in range(nchunk_t):
                            sl = slice(ch * r, (ch + 1) * r)
                            pe_sum(0, rv(b, sl))
                            pe_sum(1, dv(sl))
                    for cc in range((rdt + rc - 1) // rc):
                        rct = min(rc, rdt - cc * rc)
                        cs = slice(cc * rc, cc * rc + rct)
                        dk = midp.tile([P, rct], dt.bfloat16, tag=f"dk{b}",
                                       name=f"dk{b}")
                        nc.vector.tensor_tensor(dk[:], dv(cs), mv(b, cs),
                                                OP.mult)
                        dms = midp.tile([P, rct], dt.bfloat16, tag=f"dms{b}",
                                        name=f"dms{b}")
                        on_act = dm_insts[dminst[0]]
                        dminst[0] += 1
                        if on_act:
                            dc = dmcol[0]
                            dmcol[0] += 1
                            nc.scalar.activation(
                                dms[:], dk[:], AT.Square,
                                accum_out=acc[:, nmask + dc:nmask + dc + 1])
                        else:
                            nc.vector.tensor_tensor(dms[:], dk[:], dk[:],
                                                    OP.mult)
                        for ch in range(rct // r):
                            sl = slice(cc * rc + ch * r, cc * rc + (ch + 1) * r)
                            sl2 = slice(ch * r, (ch + 1) * r)
                            if not early_stats:
                                pe_sum(0, rv(b, sl))
                                pe_sum(1, dv(sl))
                            if not on_act:
                                pe_sum(2, dms[:, sl2])

            # ship raw per-column stat partials; the host sums 1536 floats
            sums_s = outp.tile([1, NSEG * r], dt.float32)
            nc.scalar.copy(sums_s[:], stats[:])
            nc.sync.dma_start(sums_d[:], sums_s[:])
            nc.sync.dma_start(acc_d[:], acc[:])

    nc.compile()
    meta = {"nmask": nmask, "seq_dtype": seq_dtype, "r": r}
    return nc, meta


def make_consts():
    import ml_dtypes
    bf16 = ml_dtypes.bfloat16
    # tri[k, j] = 1 if k < j  (strictly-lower prefix: C[j] = # zeros before j)
    tri = np.triu(np.ones((P, P), dtype=np.float32), 1).astype(bf16)
    onesm = np.ones((P, P), dtype=bf16)
    ones = np.ones((P, 1), dtype=bf16)
    return tri, onesm, ones


def prep_shards(sample_seq, sample_value, sample_reward, seq_dtype="u8"):
    """Host-side shard prep: batch-shard 8 ways, transpose to [S, cols]."""
    import ml_dtypes
    bf16 = ml_dtypes.bfloat16
    seq_np = np.uint8 if seq_dtype == "u8" else bf16
    seq_bf = np.asarray(sample_seq).astype(seq_np)        # values in [0, 20)
    rew_bf = np.asarray(sample_reward).astype(bf16)
    val_bf = np.asarray(sample_value).astype(bf16)

    tri, onesm, ones = make_consts()
    in_maps = []
    for c in range(N_CORES):
        lo, hi = c * COLS, (c + 1) * COLS
        in_maps.append({
            "seq": np.ascontiguousarray(seq_bf[lo:hi].T),
            "rew": np.ascontiguousarray(rew_bf[lo:hi].T),
            "val": np.ascontiguousarray(val_bf[lo:hi].T),
            "tri": tri,
            "onesm": onesm,
            "ones": ones,
        })
    return in_maps


def combine(parts, meta):
    """parts: per-core dicts with 'sums' [1, 3*R] (r/d/dm per-column partials)
    and 'acc' [P, nmask] mask sums."""
    sum_r = sum_mask = sum_d = sum_dm = 0.0
    rr = meta["r"]
    for p in parts:
        sums = np.asarray(p["sums"], dtype=np.float64)[0]
        sum_r += sums[0:rr].sum()
        sum_d += sums[rr:2 * rr].sum()
        acc = np.asarray(p["acc"], dtype=np.float64)
        sum_dm += sums[2 * rr:].sum() + acc[:, meta["nmask"]:].sum()
        sum_mask += acc[:, :meta["nmask"]].sum()
    n = float(B) * float(S)
    return np.array([sum_dm / sum_mask, sum_d / n, sum_r / n], dtype=np.float32)


def run(sample_seq, sample_value, sample_reward, trace=False, build_kwargs=None,
        **kwargs):
    from concourse.bass_utils import run_bass_kernel_spmd

    key = tuple(sorted((build_kwargs or {}).items()))
    if key not in _cache:
        _cache[key] = build_nc(COLS, **(build_kwargs or {}))
    nc, meta = _cache[key]

    in_maps = prep_shards(sample_seq, sample_value, sample_reward,
                          seq_dtype=meta["seq_dtype"])
    res = run_bass_kernel_spmd(nc, in_maps, core_ids=list(range(N_CORES)),
                               trace=trace, **kwargs)
    return combine(res.results, meta), res


def kernel(sample_seq, sample_value, sample_reward):
    out, _ = run(sample_seq, sample_value, sample_reward)
    return out
